# revision 18
# baseline (speedup 1.0000x reference)
"""Trainium2 Bass kernel for nn_AttentionLayer (B=16, V=1024, D=512, H=8, MAXHOP=8).

Sharding: 4 head-groups x 2 batch-groups. Core c = 2*hg + bg handles heads
{2hg, 2hg+1} for batches bg*8..bg*8+7. The relative-position table is built
factored: w = exp(rpe)[hop] so that P' = exp(S) * w. Core (hg, bg) builds
head (2hg+bg)'s w-table on DVE and a 2-replica AllGather exchanges the
pair's tables. Output partials (2 heads summed via one K=128 out-proj
matmul) are written fp32 straight from PSUM to DRAM and summed on host.

v2 changes vs baseline (329us):
  - x streamed in 512-token chunks (frees 56KB/partition of SBUF)
  - P' = P*w and w-build accumulation use scalar_tensor_tensor /
    tensor_scalar (InstTensorScalarPtr, 4x DVE mode) instead of
    tensor_tensor (2x only)
  - output projection writes PSUM->DRAM fp32 directly (no copy insts),
    interleaved into the next batch's jt-loop (shared PSUM slots)
  - reciprocal on a [128, 8] reshape of the denominator row (DMA
    round-trip) instead of a 1-partition [1, 1024] op
  - softmax normalization mult on GpSimd (pool) from an SBUF copy of
    PSUM (DMA'd), freeing DVE
  - q/k PSUM->SBUF casts on DVE, v cast on ACT (load balance); v cast
    batched 4 token-tiles per instruction
  - q scale folded into Wq on host
"""

import numpy as np

import concourse.bass as bass
import concourse.tile as tile
from concourse import bacc, mybir

FP32 = mybir.dt.float32
BF16 = mybir.dt.bfloat16
AOT = mybir.AluOpType
ACTF = mybir.ActivationFunctionType

N_CORES = 8
B, V, D, H, NHOP = 16, 1024, 512, 8, 9
HG, BG = 4, 2                 # head groups x batch groups
HPC = H // HG                 # heads per core (=2)
BPC = B // BG                 # batches per core (=8)
HD = D // H                   # head dim (=64)
DH = HPC * HD                 # head-pair dims (=128)
T = BPC * V                   # tokens per core (=8192)
NJT = V // 128                # key-position tiles (=8)
NTT = T // 128                # token tiles (=64)
NTC = T // 512                # proj token chunks (=16)
DCH = D // 128                # contraction chunks (=4)

USE_POOL_NORM = False         # GpSimd cannot run TensorScalarPtr (ISA check)


def build_graph(tc, out_d, ins, core_groups, dbg=None):
    from contextlib import ExitStack

    ctx = ExitStack()
    nc = tc.nc

    xT_d, wq_d, wk_d, wv_d = ins["xT"], ins["WqTc"], ins["WkTc"], ins["WvTc"]
    woth_d, hop_d, rpeb_d = ins["WoTh"], ins["hopT"], ins["rpeb"]

    consts = ctx.enter_context(tc.tile_pool(name="consts", bufs=1))
    persist = ctx.enter_context(tc.tile_pool(name="persist", bufs=1))
    dram = ctx.enter_context(tc.tile_pool(name="dram", bufs=1, space="DRAM"))
    dram_rec = ctx.enter_context(tc.tile_pool(name="dram_rec", bufs=6,
                                              space="DRAM"))

    # ---- weight loads (issued first so projections can start ASAP) ------
    wq_sb = consts.tile([128, DCH, DH], BF16, name="wq_sb")
    wk_sb = consts.tile([128, DCH, DH], BF16, name="wk_sb")
    wv_sb = consts.tile([128, DCH, DH], BF16, name="wv_sb")
    for d_ap, sb in ((wq_d, wq_sb), (wk_d, wk_sb), (wv_d, wv_sb)):
        for kc in range(DCH):
            nc.sync.dma_start(sb[:, kc, :], d_ap[kc * 128:(kc + 1) * 128, :])
    wof = consts.tile([DH, D], FP32, name="wof")
    nc.sync.dma_start(wof[:], woth_d)

    # ---- exp(rpe) row -> broadcast [128, NHOP] ---------------------------
    rpe_sb = consts.tile([1, NHOP], FP32, name="rpe_sb")
    nc.sync.dma_start(rpe_sb[:], rpeb_d)
    w9 = consts.tile([1, NHOP], FP32, name="w9")
    nc.scalar.activation(w9[:], rpe_sb[:], ACTF.Exp)
    w9_dram = dram.tile([1, NHOP], FP32, name="w9_dram")
    nc.sync.dma_start(w9_dram[:], w9[:])
    wv9 = consts.tile([128, NHOP], FP32, name="wv9")
    nc.sync.dma_start(wv9[:], w9_dram[:].broadcast_to([128, NHOP]))

    # ---- persistent tensors ----------------------------------------------
    qT = persist.tile([DH, T], BF16, name="qT")
    kT = persist.tile([DH, T], BF16, name="kT")
    vt = persist.tile([128, NTT, HPC, HD + 1], BF16, name="vt")
    nc.vector.memset(vt[:, :, :, HD:HD + 1], 1.0)
    w_A = persist.tile([128, NJT * V], BF16, name="w_A")
    w_B = persist.tile([128, NJT * V], BF16, name="w_B")
    att_all = persist.tile([DH, T], BF16, name="att_all")
    woth = persist.tile([DH, D], BF16, name="woth")
    nc.scalar.copy(woth[:], wof[:])

    own_dram = dram.tile([V, V], BF16, name="own_dram")
    gathered = [dram.tile([HPC, V // 2, V], BF16, name=f"gathered{i}")
                for i in range(2)]

    # ---- hop load (after weights; build needs it) ------------------------
    ctx_load = ExitStack()
    lpool = ctx_load.enter_context(tc.tile_pool(name="lpool", bufs=1))
    hop_sb = lpool.tile([128, NJT * V], BF16, name="hop_sb")
    for jt in range(NJT):
        nc.sync.dma_start(hop_sb[:, jt * V:(jt + 1) * V],
                          hop_d[jt * 128:(jt + 1) * 128, :])

    # ---- projections (x streamed per 512-token chunk) --------------------
    ctx_proj = ExitStack()
    xpool = ctx_proj.enter_context(tc.tile_pool(name="xpool", bufs=3))
    ps_proj = ctx_proj.enter_context(
        tc.tile_pool(name="ps_proj", bufs=2, space="PSUM"))

    for tcn in range(NTC):
        tsl = slice(tcn * 512, (tcn + 1) * 512)
        xc = xpool.tile([128, DCH, 512], BF16, name="xc", tag="xc")
        for kc in range(DCH):
            nc.sync.dma_start(xc[:, kc, :],
                              xT_d[kc * 128:(kc + 1) * 128, tsl])
        ps_q = ps_proj.tile([DH, 512], FP32, name="ps_q", tag="ps_q")
        for kc in range(DCH):
            nc.tensor.matmul(ps_q[:], wq_sb[:, kc, :], xc[:, kc, :],
                             start=(kc == 0), stop=(kc == DCH - 1))
        nc.scalar.copy(qT[:, tsl], ps_q[:])
        ps_k = ps_proj.tile([DH, 512], FP32, name="ps_k", tag="ps_k")
        for kc in range(DCH):
            nc.tensor.matmul(ps_k[:], wk_sb[:, kc, :], xc[:, kc, :],
                             start=(kc == 0), stop=(kc == DCH - 1))
        nc.scalar.copy(kT[:, tsl], ps_k[:])
        # v token-major, 4 token-tiles packed in one PSUM tile / one copy
        ps_v = ps_proj.tile([128, 512], FP32, name="ps_v", tag="ps_v")
        for s in range(4):
            ssl = slice(s * 128, (s + 1) * 128)
            for kc in range(DCH):
                nc.tensor.matmul(ps_v[:, ssl], xc[:, kc, ssl],
                                 wv_sb[:, kc, :],
                                 start=(kc == 0), stop=(kc == DCH - 1))
        nc.scalar.copy(
            vt[:, tcn * 4:(tcn + 1) * 4, :, 0:HD],
            ps_v[:].rearrange("p (t h d) -> p t h d", t=4, h=HPC))

    ctx_proj.close()

    # ---- w-table build (DVE, 4-key-tile-wide ops) + pair AllGather -------
    # Each op covers half the table ([128, 4V]): fewer instructions, same
    # element throughput at DVE 4x.
    bpool = ctx_load.enter_context(tc.tile_pool(name="bpool", bufs=1))
    HNJ = NJT // 2
    for half in range(2):
        hsl = hop_sb[:, half * HNJ * V:(half + 1) * HNJ * V]
        acc = bpool.tile([128, HNJ * V], BF16, name="bacc", tag="bacc")
        nc.vector.tensor_scalar(
            acc[:], hsl, 0.0, wv9[:, 0:1],
            AOT.is_equal, AOT.mult)
        for m in range(1, NHOP):
            term = bpool.tile([128, HNJ * V], BF16, name="bterm", tag="bterm")
            nc.vector.tensor_scalar(
                term[:], hsl, float(m), wv9[:, m:m + 1],
                AOT.is_equal, AOT.mult)
            nc.vector.scalar_tensor_tensor(
                acc[:], term[:], 0.0, acc[:], AOT.add, AOT.add)
        for jr in range(HNJ):
            jt = half * HNJ + jr
            nc.sync.dma_start(own_dram[jt * 128:(jt + 1) * 128, :],
                              acc[:, jr * V:(jr + 1) * V])
        nc.gpsimd.collective_compute(
            "AllGather",
            AOT.bypass,
            replica_groups=core_groups,
            ins=[own_dram[half * (V // 2):(half + 1) * (V // 2), :].opt()],
            outs=[gathered[half][:].opt()],
        )
    for jt in range(NJT):
        half, jr = divmod(jt, NJT // 2)
        nc.sync.dma_start(w_A[:, jt * V:(jt + 1) * V],
                          gathered[half][0, jr * 128:(jr + 1) * 128, :])
        nc.sync.dma_start(w_B[:, jt * V:(jt + 1) * V],
                          gathered[half][1, jr * 128:(jr + 1) * 128, :])
    ctx_load.close()

    # ---- attention + interleaved output projection -----------------------
    ctx_att = ExitStack()
    ps_pool = ctx_att.enter_context(
        tc.tile_pool(name="ps", bufs=2, space="PSUM"))
    ps_att_pool = ctx_att.enter_context(
        tc.tile_pool(name="ps_att", bufs=2, space="PSUM"))
    p_pool = ctx_att.enter_context(tc.tile_pool(name="pp", bufs=12))
    p2_pool = ctx_att.enter_context(tc.tile_pool(name="p2p", bufs=4))
    attr_pool = ctx_att.enter_context(tc.tile_pool(name="attrp", bufs=3))
    rec_pool = ctx_att.enter_context(tc.tile_pool(name="recp", bufs=3))
    o_pool = ctx_att.enter_context(tc.tile_pool(name="op", bufs=4))

    norm_engine = nc.gpsimd if USE_POOL_NORM else nc.vector

    def emit_out(bb, pair):
        # two token-tiles per PSUM tile / copy / DMA (amortize overheads)
        it = 2 * pair
        ps_o = ps_pool.tile([128, 2, D], FP32, name="ps_o", tag="ps")
        for i in range(2):
            nc.tensor.matmul(
                ps_o[:, i, :],
                att_all[:, bb * V + (it + i) * 128:bb * V + (it + i + 1) * 128],
                woth[:], start=True, stop=True)
        o_sb = o_pool.tile([128, 2, D], BF16, name="o_sb", tag="o_sb")
        nc.vector.tensor_copy(o_sb[:], ps_o[:])
        for i in range(2):
            nc.sync.dma_start(out_d[bb, (it + i) * 128:(it + i + 1) * 128, :],
                              o_sb[:, i, :])

    for b in range(BPC):
        t0 = b * V
        ps_att = {}
        for h in range(HPC):
            ps_att[h] = ps_att_pool.tile([HD + 1, V], FP32,
                                         name=f"ps_att{h}", tag="ps_att")
        p2s = {}

        def emit_pv(jt, b=b, ps_att=ps_att, p2s=p2s):
            for h in range(HPC):
                p2 = p2s.pop((jt, h))
                for ic in range(2):
                    isl = slice(ic * 512, (ic + 1) * 512)
                    nc.tensor.matmul(ps_att[h][:, isl],
                                     vt[:, b * NJT + jt, h, :], p2[:, isl],
                                     start=(jt == 0), stop=(jt == NJT - 1))

        for jt in range(NJT):
            jsl = slice(t0 + jt * 128, t0 + (jt + 1) * 128)
            ps_sA = ps_pool.tile([128, V], FP32, name="ps_sA", tag="ps")
            ps_sB = ps_pool.tile([128, V], FP32, name="ps_sB", tag="ps")
            for sc in range(2):
                csl_ = slice(sc * 512, (sc + 1) * 512)
                ssl = slice(t0 + sc * 512, t0 + (sc + 1) * 512)
                nc.tensor.matmul(ps_sA[:, csl_], kT[0:HD, jsl], qT[0:HD, ssl],
                                 start=True, stop=True)
                nc.tensor.matmul(ps_sB[:, csl_], kT[HD:DH, jsl],
                                 qT[HD:DH, ssl], start=True, stop=True)
            for h, ps_s, w_t in ((0, ps_sA, w_A), (1, ps_sB, w_B)):
                p_sb = p_pool.tile([128, V], BF16, name="p_sb", tag="p")
                nc.scalar.activation(p_sb[:], ps_s[:], ACTF.Exp)
                p2 = p2_pool.tile([128, V], BF16, name="p2", tag="p2")
                nc.vector.scalar_tensor_tensor(
                    p2[:], p_sb[:], 1.0, w_t[:, jt * V:(jt + 1) * V],
                    AOT.mult, AOT.mult)
                p2s[(jt, h)] = p2
            # software-pipelined PV: consume the previous jt's P'
            if jt > 0:
                emit_pv(jt - 1)
            # out-proj of the previous batch rides the tail of this one
            if b > 0 and jt >= 6:
                for pair in range(2 * (jt - 6), 2 * (jt - 5)):
                    emit_out(b - 1, pair)
        emit_pv(NJT - 1)

        # ---- drain PSUM, then denominators + normalization ---------------
        for h in range(HPC):
            # single fast drain of the accumulated [att; den] tile to SBUF
            attr = attr_pool.tile([HD + 1, V], BF16, name="attr", tag="attr")
            nc.vector.tensor_copy(attr[:], ps_att[h][:])
            # den row -> DRAM -> [128, 8] -> reciprocal -> broadcast row
            dden = dram_rec.tile([1, V], BF16, name="dden", tag="dden")
            nc.sync.dma_start(dden[:], attr[HD:HD + 1, :])
            den128 = rec_pool.tile([128, 8], BF16, name="den128",
                                   tag="den128")
            nc.gpsimd.dma_start(
                den128[:], dden[:].rearrange("a (p c) -> (a p) c", p=128))
            den128f = rec_pool.tile([128, 8], FP32, name="den128f",
                                    tag="den128f")
            nc.vector.tensor_copy(den128f[:], den128[:])
            r128 = rec_pool.tile([128, 8], FP32, name="r128", tag="r128")
            nc.vector.reciprocal_approx_fast(r128[:], den128f[:])
            r128b = rec_pool.tile([128, 8], BF16, name="r128b", tag="r128b")
            nc.vector.tensor_copy(r128b[:], r128[:])
            rrec = dram_rec.tile([1, V], BF16, name="rrec", tag="rrec")
            nc.gpsimd.dma_start(
                rrec[:].rearrange("a (p c) -> (a p) c", p=128), r128b[:])
            rbc = rec_pool.tile([HD, V], BF16, name="rbc", tag="rbc")
            nc.gpsimd.dma_start(rbc[:], rrec[:].broadcast_to([HD, V]))
            norm_engine.scalar_tensor_tensor(
                att_all[h * HD:(h + 1) * HD, t0:t0 + V],
                attr[0:HD, :], 1.0, rbc[:], AOT.mult, AOT.mult)

    for pair in range(NJT // 2):
        emit_out(BPC - 1, pair)

    if dbg is not None:
        for nm, t in (("qT", qT), ("kT", kT), ("w_A", w_A), ("w_B", w_B),
                      ("att", att_all)):
            nslices = t.shape[1] // 4096 if t.shape[1] >= 4096 else 1
            for sidx in range(nslices):
                sl = slice(sidx * 4096, (sidx + 1) * 4096)
                nc.sync.dma_start(dbg[nm][:, sl], t[:, sl])

    ctx_att.close()
    ctx.close()


# --------------------------------------------------------------------------
# Host side
# --------------------------------------------------------------------------

def _bf16(a):
    import ml_dtypes
    return np.ascontiguousarray(a.astype(ml_dtypes.bfloat16))


def shard_inputs(x, Wq, Wk, Wv, Wo, bo, rpe, hop_matrix):
    x = np.asarray(x, np.float32)
    scale = 1.0 / np.float32(np.sqrt(HD))
    WqT = np.asarray(Wq, np.float32).T * scale
    WkT = np.asarray(Wk, np.float32).T
    WvT = np.asarray(Wv, np.float32).T
    WoT = np.asarray(Wo, np.float32).T
    hopT = np.asarray(hop_matrix).T.astype(np.float32)
    rpe = np.asarray(rpe, np.float32)
    in_maps = []
    for c in range(N_CORES):
        hg, bg = c // BG, c % BG
        csl = slice(hg * DH, (hg + 1) * DH)
        xs = x[bg * BPC:(bg + 1) * BPC].reshape(T, D).T
        head_built = HPC * hg + bg
        in_maps.append({
            "xT": _bf16(xs),
            "WqTc": _bf16(WqT[:, csl]),
            "WkTc": _bf16(WkT[:, csl]),
            "WvTc": _bf16(WvT[:, csl]),
            "WoTh": np.ascontiguousarray(WoT[csl, :]),
            "hopT": _bf16(hopT),
            "rpeb": np.ascontiguousarray(rpe[head_built:head_built + 1, :]),
        })
    return in_maps


def unshard_output(results, bo):
    bo = np.asarray(bo, np.float32)
    outs = []
    for bg in range(BG):
        acc = np.zeros((BPC, V, D), np.float32)
        for hg in range(HG):
            acc += results[hg * BG + bg]["out"].astype(np.float32)
        outs.append(acc + bo)
    return np.concatenate(outs, axis=0)


_CACHE = {}


def _get_compiled():
    if "nc" in _CACHE:
        return _CACHE["nc"]
    nc = bacc.Bacc("TRN2", target_bir_lowering=False, debug=False,
                   num_devices=N_CORES)
    ins = {
        "xT": nc.dram_tensor("xT", [D, T], BF16, kind="ExternalInput").ap(),
        "WqTc": nc.dram_tensor("WqTc", [D, DH], BF16,
                               kind="ExternalInput").ap(),
        "WkTc": nc.dram_tensor("WkTc", [D, DH], BF16,
                               kind="ExternalInput").ap(),
        "WvTc": nc.dram_tensor("WvTc", [D, DH], BF16,
                               kind="ExternalInput").ap(),
        "WoTh": nc.dram_tensor("WoTh", [DH, D], FP32,
                               kind="ExternalInput").ap(),
        "hopT": nc.dram_tensor("hopT", [V, V], BF16,
                               kind="ExternalInput").ap(),
        "rpeb": nc.dram_tensor("rpeb", [1, NHOP], FP32,
                               kind="ExternalInput").ap(),
    }
    out = nc.dram_tensor("out", [BPC, V, D], BF16,
                         kind="ExternalOutput").ap()
    core_groups = [[2 * g, 2 * g + 1] for g in range(HG)]
    import os
    dbg = None
    if os.environ.get("KBG_DEBUG"):
        dbg = {
            "qT": nc.dram_tensor("dbg_qT", [DH, T], BF16,
                                 kind="ExternalOutput").ap(),
            "kT": nc.dram_tensor("dbg_kT", [DH, T], BF16,
                                 kind="ExternalOutput").ap(),
            "w_A": nc.dram_tensor("dbg_w_A", [128, NJT * V], BF16,
                                  kind="ExternalOutput").ap(),
            "w_B": nc.dram_tensor("dbg_w_B", [128, NJT * V], BF16,
                                  kind="ExternalOutput").ap(),
            "att": nc.dram_tensor("dbg_att", [DH, T], BF16,
                                  kind="ExternalOutput").ap(),
        }
    with tile.TileContext(nc) as tc:
        build_graph(tc, out, ins, core_groups, dbg)
    nc.compile()
    _CACHE["nc"] = nc
    return nc


def kernel(x, Wq, Wk, Wv, Wo, bo, rpe, hop_matrix):
    from concourse.bass_utils import run_bass_kernel_spmd

    nc = _get_compiled()
    in_maps = shard_inputs(x, Wq, Wk, Wv, Wo, bo, rpe, hop_matrix)
    res = run_bass_kernel_spmd(nc, in_maps, core_ids=list(range(N_CORES)))
    return unshard_output(res.results, bo)


# revision 19
# speedup vs baseline: 1.4631x; 1.4631x over previous
"""Trainium2 Bass kernel for nn_AttentionLayer (B=16, V=1024, D=512, H=8, MAXHOP=8).

Sharding: 4 head-groups x 2 batch-groups. Core c = 2*hg + bg handles heads
{2hg, 2hg+1} for batches bg*8..bg*8+7. The relative-position factor
w = exp(rpe)[hop] (a 9-entry-table gather over the replicated hop matrix)
is materialized host-side during input sharding, in the transposed
[key, query] layout each core consumes, so that on device
P' = exp(S) * w with no bias-inject matmuls and no table-build phase.

Per-core device math (transposed-score layout):
  qT/kT = (W @ x^T) per head-pair [128, tokens] bf16 (q pre-scaled 1/sqrt(hd))
  S_h[j,i] = k_j . q_i            (K=64 matmuls, heads at PE rows 0-63/64-127)
  P_h = exp(S_h)                  (ScalarE, PSUM -> SBUF bf16)
  P'_h = P_h * w_h[jt]            (DVE bf16 2x)
  att_T[d,i] (+denom row) = [v|1]^T @ P'_h   (ones-augmented V, M=65)
  attr = PSUM drain (bf16), denom -> [128,8] reciprocal -> row broadcast
  att = attr * recip              (DVE 2x)
  out_part = [att_A; att_B] @ [WoT_A; WoT_B]  (K=128 matmul, 2 tiles/PSUM)
Output partials (2 heads summed) are bf16; host sums the 4 head-groups
and adds bo.
"""

import numpy as np

import concourse.bass as bass
import concourse.tile as tile
from concourse import bacc, mybir

FP32 = mybir.dt.float32
BF16 = mybir.dt.bfloat16
AOT = mybir.AluOpType
ACTF = mybir.ActivationFunctionType

N_CORES = 8
B, V, D, H, NHOP = 16, 1024, 512, 8, 9
HG, BG = 4, 2                 # head groups x batch groups
HPC = H // HG                 # heads per core (=2)
BPC = B // BG                 # batches per core (=8)
HD = D // H                   # head dim (=64)
DH = HPC * HD                 # head-pair dims (=128)
T = BPC * V                   # tokens per core (=8192)
NJT = V // 128                # key-position tiles (=8)
NTT = T // 128                # token tiles (=64)
NTC = T // 512                # proj token chunks (=16)
DCH = D // 128                # contraction chunks (=4)


def build_graph(tc, out_d, ins, dbg=None):
    from contextlib import ExitStack

    ctx = ExitStack()
    nc = tc.nc

    xT_d, wq_d, wk_d, wv_d = ins["xT"], ins["WqTc"], ins["WkTc"], ins["WvTc"]
    woth_d, wA_d, wB_d = ins["WoTh"], ins["wA"], ins["wB"]

    consts = ctx.enter_context(tc.tile_pool(name="consts", bufs=1))
    persist = ctx.enter_context(tc.tile_pool(name="persist", bufs=1))
    dram_rec = ctx.enter_context(tc.tile_pool(name="dram_rec", bufs=6,
                                              space="DRAM"))

    # ---- weight / table loads (issued first) -----------------------------
    wq_sb = consts.tile([128, DCH, DH], BF16, name="wq_sb")
    wk_sb = consts.tile([128, DCH, DH], BF16, name="wk_sb")
    wv_sb = consts.tile([128, DCH, DH], BF16, name="wv_sb")
    for d_ap, sb in ((wq_d, wq_sb), (wk_d, wk_sb), (wv_d, wv_sb)):
        for kc in range(DCH):
            nc.sync.dma_start(sb[:, kc, :], d_ap[kc * 128:(kc + 1) * 128, :])
    wof = consts.tile([DH, D], FP32, name="wof")
    nc.sync.dma_start(wof[:], woth_d)

    # ---- persistent tensors ----------------------------------------------
    qT = persist.tile([DH, T], BF16, name="qT")
    kT = persist.tile([DH, T], BF16, name="kT")
    vt = persist.tile([128, NTT, HPC, HD + 1], BF16, name="vt")
    nc.vector.memset(vt[:, :, :, HD:HD + 1], 1.0)
    w_A = persist.tile([128, NJT * V], BF16, name="w_A")
    w_B = persist.tile([128, NJT * V], BF16, name="w_B")
    att_all = persist.tile([DH, T], BF16, name="att_all")
    woth = persist.tile([DH, D], BF16, name="woth")
    nc.scalar.copy(woth[:], wof[:])

    # relative-position tables (host-built, transposed layout)
    for jt in range(NJT):
        nc.sync.dma_start(w_A[:, jt * V:(jt + 1) * V],
                          wA_d[jt * 128:(jt + 1) * 128, :])
        nc.sync.dma_start(w_B[:, jt * V:(jt + 1) * V],
                          wB_d[jt * 128:(jt + 1) * 128, :])

    # ---- projections (x streamed per 512-token chunk) --------------------
    ctx_proj = ExitStack()
    xpool = ctx_proj.enter_context(tc.tile_pool(name="xpool", bufs=3))
    ps_proj = ctx_proj.enter_context(
        tc.tile_pool(name="ps_proj", bufs=2, space="PSUM"))

    for tcn in range(NTC):
        tsl = slice(tcn * 512, (tcn + 1) * 512)
        xc = xpool.tile([128, DCH, 512], BF16, name="xc", tag="xc")
        for kc in range(DCH):
            nc.sync.dma_start(xc[:, kc, :],
                              xT_d[kc * 128:(kc + 1) * 128, tsl])
        ps_q = ps_proj.tile([DH, 512], FP32, name="ps_q", tag="ps_q")
        for kc in range(DCH):
            nc.tensor.matmul(ps_q[:], wq_sb[:, kc, :], xc[:, kc, :],
                             start=(kc == 0), stop=(kc == DCH - 1))
        nc.vector.tensor_copy(qT[:, tsl], ps_q[:])
        ps_k = ps_proj.tile([DH, 512], FP32, name="ps_k", tag="ps_k")
        for kc in range(DCH):
            nc.tensor.matmul(ps_k[:], wk_sb[:, kc, :], xc[:, kc, :],
                             start=(kc == 0), stop=(kc == DCH - 1))
        nc.vector.tensor_copy(kT[:, tsl], ps_k[:])
        # v token-major, 4 token-tiles packed in one PSUM tile / one copy
        ps_v = ps_proj.tile([128, 512], FP32, name="ps_v", tag="ps_v")
        for s in range(4):
            ssl = slice(s * 128, (s + 1) * 128)
            for kc in range(DCH):
                nc.tensor.matmul(ps_v[:, ssl], xc[:, kc, ssl],
                                 wv_sb[:, kc, :],
                                 start=(kc == 0), stop=(kc == DCH - 1))
        nc.scalar.copy(
            vt[:, tcn * 4:(tcn + 1) * 4, :, 0:HD],
            ps_v[:].rearrange("p (t h d) -> p t h d", t=4, h=HPC))

    ctx_proj.close()

    # ---- attention + interleaved output projection -----------------------
    ctx_att = ExitStack()
    ps_pool = ctx_att.enter_context(
        tc.tile_pool(name="ps", bufs=2, space="PSUM"))
    ps_att_pool = ctx_att.enter_context(
        tc.tile_pool(name="ps_att", bufs=2, space="PSUM"))
    p_pool = ctx_att.enter_context(tc.tile_pool(name="pp", bufs=10))
    p2_pool = ctx_att.enter_context(tc.tile_pool(name="p2p", bufs=4))
    attr_pool = ctx_att.enter_context(tc.tile_pool(name="attrp", bufs=3))
    rec_pool = ctx_att.enter_context(tc.tile_pool(name="recp", bufs=3))
    o_pool = ctx_att.enter_context(tc.tile_pool(name="op", bufs=4))

    def emit_out(bb, pair):
        # two token-tiles per PSUM tile / copy (amortize overheads)
        it = 2 * pair
        ps_o = ps_pool.tile([128, 2, D], FP32, name="ps_o", tag="ps")
        for i in range(2):
            nc.tensor.matmul(
                ps_o[:, i, :],
                att_all[:, bb * V + (it + i) * 128:bb * V + (it + i + 1) * 128],
                woth[:], start=True, stop=True)
        o_sb = o_pool.tile([128, 2, D], BF16, name="o_sb", tag="o_sb")
        nc.vector.tensor_copy(o_sb[:], ps_o[:])
        for i in range(2):
            nc.sync.dma_start(out_d[bb, (it + i) * 128:(it + i + 1) * 128, :],
                              o_sb[:, i, :])

    for b in range(BPC):
        t0 = b * V
        ps_att = {}
        for h in range(HPC):
            ps_att[h] = ps_att_pool.tile([HD + 1, V], FP32,
                                         name=f"ps_att{h}", tag="ps_att")
        p2s = {}

        def emit_pv(jt, b=b, ps_att=ps_att, p2s=p2s):
            for h in range(HPC):
                p2 = p2s.pop((jt, h))
                for ic in range(2):
                    isl = slice(ic * 512, (ic + 1) * 512)
                    nc.tensor.matmul(ps_att[h][:, isl],
                                     vt[:, b * NJT + jt, h, :], p2[:, isl],
                                     start=(jt == 0), stop=(jt == NJT - 1))

        for jt in range(NJT):
            jsl = slice(t0 + jt * 128, t0 + (jt + 1) * 128)
            ps_sA = ps_pool.tile([128, V], FP32, name="ps_sA", tag="ps")
            ps_sB = ps_pool.tile([128, V], FP32, name="ps_sB", tag="ps")
            for sc in range(2):
                csl_ = slice(sc * 512, (sc + 1) * 512)
                ssl = slice(t0 + sc * 512, t0 + (sc + 1) * 512)
                nc.tensor.matmul(ps_sA[:, csl_], kT[0:HD, jsl], qT[0:HD, ssl],
                                 start=True, stop=True)
                nc.tensor.matmul(ps_sB[:, csl_], kT[HD:DH, jsl],
                                 qT[HD:DH, ssl], start=True, stop=True)
            for h, ps_s, w_t in ((0, ps_sA, w_A), (1, ps_sB, w_B)):
                p_sb = p_pool.tile([128, V], BF16, name="p_sb", tag="p")
                nc.scalar.activation(p_sb[:], ps_s[:], ACTF.Exp)
                p2 = p2_pool.tile([128, V], BF16, name="p2", tag="p2")
                nc.vector.tensor_tensor(p2[:], p_sb[:],
                                        w_t[:, jt * V:(jt + 1) * V],
                                        AOT.mult)
                p2s[(jt, h)] = p2
            # software-pipelined PV: consume the previous jt's P'
            if jt > 0:
                emit_pv(jt - 1)
            # out-proj of the previous batch rides the tail of this one
            if b > 0 and jt >= 6:
                for pair in range(2 * (jt - 6), 2 * (jt - 5)):
                    emit_out(b - 1, pair)
        emit_pv(NJT - 1)

        # ---- drain PSUM, then denominators + normalization ---------------
        for h in range(HPC):
            # single fast drain of the accumulated [att; den] tile to SBUF
            attr = attr_pool.tile([HD + 1, V], BF16, name="attr", tag="attr")
            nc.scalar.copy(attr[:], ps_att[h][:])
            # den row -> DRAM -> [128, 8] -> reciprocal -> broadcast row
            dden = dram_rec.tile([1, V], BF16, name="dden", tag="dden")
            nc.sync.dma_start(dden[:], attr[HD:HD + 1, :])
            den128 = rec_pool.tile([128, 8], BF16, name="den128",
                                   tag="den128")
            nc.gpsimd.dma_start(
                den128[:], dden[:].rearrange("a (p c) -> (a p) c", p=128))
            den128f = rec_pool.tile([128, 8], FP32, name="den128f",
                                    tag="den128f")
            nc.vector.tensor_copy(den128f[:], den128[:])
            r128 = rec_pool.tile([128, 8], FP32, name="r128", tag="r128")
            nc.vector.reciprocal_approx_fast(r128[:], den128f[:])
            r128b = rec_pool.tile([128, 8], BF16, name="r128b", tag="r128b")
            nc.vector.tensor_copy(r128b[:], r128[:])
            rrec = dram_rec.tile([1, V], BF16, name="rrec", tag="rrec")
            nc.gpsimd.dma_start(
                rrec[:].rearrange("a (p c) -> (a p) c", p=128), r128b[:])
            rbc = rec_pool.tile([HD, V], BF16, name="rbc", tag="rbc")
            nc.gpsimd.dma_start(rbc[:], rrec[:].broadcast_to([HD, V]))
            nc.vector.tensor_tensor(
                att_all[h * HD:(h + 1) * HD, t0:t0 + V],
                attr[0:HD, :], rbc[:], AOT.mult)

    for pair in range(NJT // 2):
        emit_out(BPC - 1, pair)

    if dbg is not None:
        for nm, t in (("qT", qT), ("kT", kT), ("w_A", w_A), ("w_B", w_B),
                      ("att", att_all)):
            nslices = t.shape[1] // 4096 if t.shape[1] >= 4096 else 1
            for sidx in range(nslices):
                sl = slice(sidx * 4096, (sidx + 1) * 4096)
                nc.sync.dma_start(dbg[nm][:, sl], t[:, sl])

    ctx_att.close()
    ctx.close()


# --------------------------------------------------------------------------
# Host side
# --------------------------------------------------------------------------

def _bf16(a):
    import ml_dtypes
    return np.ascontiguousarray(a.astype(ml_dtypes.bfloat16))


def shard_inputs(x, Wq, Wk, Wv, Wo, bo, rpe, hop_matrix):
    x = np.asarray(x, np.float32)
    scale = 1.0 / np.float32(np.sqrt(HD))
    WqT = np.asarray(Wq, np.float32).T * scale
    WkT = np.asarray(Wk, np.float32).T
    WvT = np.asarray(Wv, np.float32).T
    WoT = np.asarray(Wo, np.float32).T
    hopT = np.asarray(hop_matrix).T          # [key, query] layout
    wtab = np.exp(np.asarray(rpe, np.float32))  # [H, NHOP]
    in_maps = []
    for c in range(N_CORES):
        hg, bg = c // BG, c % BG
        csl = slice(hg * DH, (hg + 1) * DH)
        xs = x[bg * BPC:(bg + 1) * BPC].reshape(T, D).T
        hA, hB = 2 * hg, 2 * hg + 1
        in_maps.append({
            "xT": _bf16(xs),
            "WqTc": _bf16(WqT[:, csl]),
            "WkTc": _bf16(WkT[:, csl]),
            "WvTc": _bf16(WvT[:, csl]),
            "WoTh": np.ascontiguousarray(WoT[csl, :]),
            "wA": _bf16(wtab[hA][hopT]),
            "wB": _bf16(wtab[hB][hopT]),
        })
    return in_maps


def unshard_output(results, bo):
    bo = np.asarray(bo, np.float32)
    outs = []
    for bg in range(BG):
        acc = np.zeros((BPC, V, D), np.float32)
        for hg in range(HG):
            acc += results[hg * BG + bg]["out"].astype(np.float32)
        outs.append(acc + bo)
    return np.concatenate(outs, axis=0)


_CACHE = {}


def _get_compiled():
    if "nc" in _CACHE:
        return _CACHE["nc"]
    nc = bacc.Bacc("TRN2", target_bir_lowering=False, debug=False,
                   num_devices=N_CORES)
    ins = {
        "xT": nc.dram_tensor("xT", [D, T], BF16, kind="ExternalInput").ap(),
        "WqTc": nc.dram_tensor("WqTc", [D, DH], BF16,
                               kind="ExternalInput").ap(),
        "WkTc": nc.dram_tensor("WkTc", [D, DH], BF16,
                               kind="ExternalInput").ap(),
        "WvTc": nc.dram_tensor("WvTc", [D, DH], BF16,
                               kind="ExternalInput").ap(),
        "WoTh": nc.dram_tensor("WoTh", [DH, D], FP32,
                               kind="ExternalInput").ap(),
        "wA": nc.dram_tensor("wA", [V, V], BF16, kind="ExternalInput").ap(),
        "wB": nc.dram_tensor("wB", [V, V], BF16, kind="ExternalInput").ap(),
    }
    out = nc.dram_tensor("out", [BPC, V, D], BF16,
                         kind="ExternalOutput").ap()
    import os
    dbg = None
    if os.environ.get("KBG_DEBUG"):
        dbg = {
            "qT": nc.dram_tensor("dbg_qT", [DH, T], BF16,
                                 kind="ExternalOutput").ap(),
            "kT": nc.dram_tensor("dbg_kT", [DH, T], BF16,
                                 kind="ExternalOutput").ap(),
            "w_A": nc.dram_tensor("dbg_w_A", [128, NJT * V], BF16,
                                  kind="ExternalOutput").ap(),
            "w_B": nc.dram_tensor("dbg_w_B", [128, NJT * V], BF16,
                                  kind="ExternalOutput").ap(),
            "att": nc.dram_tensor("dbg_att", [DH, T], BF16,
                                  kind="ExternalOutput").ap(),
        }
    with tile.TileContext(nc) as tc:
        build_graph(tc, out, ins, dbg)
    nc.compile()
    _CACHE["nc"] = nc
    return nc


def kernel(x, Wq, Wk, Wv, Wo, bo, rpe, hop_matrix):
    from concourse.bass_utils import run_bass_kernel_spmd

    nc = _get_compiled()
    in_maps = shard_inputs(x, Wq, Wk, Wv, Wo, bo, rpe, hop_matrix)
    res = run_bass_kernel_spmd(nc, in_maps, core_ids=list(range(N_CORES)))
    return unshard_output(res.results, bo)


# revision 22
# speedup vs baseline: 1.5941x; 1.0896x over previous
"""Trainium2 Bass kernel for nn_AttentionLayer (B=16, V=1024, D=512, H=8, MAXHOP=8).

Sharding: 4 head-groups x 2 batch-groups. Core c = 2*hg + bg handles heads
{2hg, 2hg+1} for batches bg*8..bg*8+7. The relative-position factor
w = exp(rpe)[hop] (a 9-entry-table gather over the replicated hop matrix)
is materialized host-side during input sharding, in the transposed
[key, query] layout each core consumes, so that on device
P' = exp(S) * w with no bias-inject matmuls and no table-build phase.

Per-core device math (transposed-score layout):
  qT/kT = (W @ x^T) per head-pair [128, tokens] bf16 (q pre-scaled 1/sqrt(hd))
  S_h[j,i] = k_j . q_i            (K=64 matmuls, heads at PE rows 0-63/64-127)
  P_h = exp(S_h)                  (ScalarE, PSUM -> SBUF bf16)
  P'_h = P_h * w_h[jt]            (DVE bf16 2x)
  att_T[d,i] (+denom row) = [v|1]^T @ P'_h   (ones-augmented V, M=65)
  attr = PSUM drain (bf16), denom -> [128,8] reciprocal -> row broadcast
  att = attr * recip              (DVE 2x)
  out_part = [att_A; att_B] @ [WoT_A; WoT_B]  (K=128 matmul, 2 tiles/PSUM)
Output partials (2 heads summed) are bf16; host sums the 4 head-groups
and adds bo.
"""

import numpy as np

import concourse.bass as bass
import concourse.tile as tile
from concourse import bacc, mybir

FP32 = mybir.dt.float32
BF16 = mybir.dt.bfloat16
AOT = mybir.AluOpType
ACTF = mybir.ActivationFunctionType

N_CORES = 8
B, V, D, H, NHOP = 16, 1024, 512, 8, 9
HG, BG = 4, 2                 # head groups x batch groups
HPC = H // HG                 # heads per core (=2)
BPC = B // BG                 # batches per core (=8)
HD = D // H                   # head dim (=64)
DH = HPC * HD                 # head-pair dims (=128)
T = BPC * V                   # tokens per core (=8192)
NJT = V // 128                # key-position tiles (=8)
NTT = T // 128                # token tiles (=64)
NTC = T // 512                # proj token chunks (=16)
DCH = D // 128                # contraction chunks (=4)


def build_graph(tc, out_d, ins, dbg=None):
    from contextlib import ExitStack

    ctx = ExitStack()
    nc = tc.nc

    xT_d, wq_d, wk_d, wv_d = ins["xT"], ins["WqTc"], ins["WkTc"], ins["WvTc"]
    woth_d, wA_d, wB_d = ins["WoTh"], ins["wA"], ins["wB"]

    consts = ctx.enter_context(tc.tile_pool(name="consts", bufs=1))
    persist = ctx.enter_context(tc.tile_pool(name="persist", bufs=1))
    dram_rec = ctx.enter_context(tc.tile_pool(name="dram_rec", bufs=6,
                                              space="DRAM"))

    # ---- weight / table loads (issued first) -----------------------------
    wq_sb = consts.tile([128, DCH, DH], BF16, name="wq_sb")
    wk_sb = consts.tile([128, DCH, DH], BF16, name="wk_sb")
    wv_sb = consts.tile([128, DCH, DH], BF16, name="wv_sb")
    for d_ap, sb in ((wq_d, wq_sb), (wk_d, wk_sb), (wv_d, wv_sb)):
        for kc in range(DCH):
            nc.sync.dma_start(sb[:, kc, :], d_ap[kc * 128:(kc + 1) * 128, :])
    wof = consts.tile([DH, D], FP32, name="wof")
    nc.sync.dma_start(wof[:], woth_d)

    # ---- persistent tensors ----------------------------------------------
    qT = persist.tile([DH, T], BF16, name="qT")
    kT = persist.tile([DH, T], BF16, name="kT")
    vt = persist.tile([128, NTT, HPC, HD + 1], BF16, name="vt")
    nc.vector.memset(vt[:, :, :, HD:HD + 1], 1.0)
    w_A = persist.tile([128, NJT * V], BF16, name="w_A")
    w_B = persist.tile([128, NJT * V], BF16, name="w_B")
    att_all = persist.tile([DH, T], BF16, name="att_all")
    woth = persist.tile([DH, D], BF16, name="woth")
    nc.scalar.copy(woth[:], wof[:])

    # relative-position tables (host-built, transposed layout)
    for jt in range(NJT):
        nc.sync.dma_start(w_A[:, jt * V:(jt + 1) * V],
                          wA_d[jt * 128:(jt + 1) * 128, :])
        nc.sync.dma_start(w_B[:, jt * V:(jt + 1) * V],
                          wB_d[jt * 128:(jt + 1) * 128, :])

    # ---- attention with per-batch interleaved projections ---------------
    ctx_att = ExitStack()
    xpool = ctx_att.enter_context(tc.tile_pool(name="xpool", bufs=4))
    ps_pool = ctx_att.enter_context(
        tc.tile_pool(name="ps", bufs=2, space="PSUM"))
    ps_att_pool = ctx_att.enter_context(
        tc.tile_pool(name="ps_att", bufs=2, space="PSUM"))
    p_pool = ctx_att.enter_context(tc.tile_pool(name="pp", bufs=10))
    p2_pool = ctx_att.enter_context(tc.tile_pool(name="p2p", bufs=4))
    attr_pool = ctx_att.enter_context(tc.tile_pool(name="attrp", bufs=3))
    rec_pool = ctx_att.enter_context(tc.tile_pool(name="recp", bufs=3))
    o_pool = ctx_att.enter_context(tc.tile_pool(name="op", bufs=4))

    def emit_proj(b):
        # q/k/v projections for batch b's two 512-token chunks
        for tcn in (2 * b, 2 * b + 1):
            tsl = slice(tcn * 512, (tcn + 1) * 512)
            xc = xpool.tile([128, DCH, 512], BF16, name="xc", tag="xc")
            for kc in range(DCH):
                nc.sync.dma_start(xc[:, kc, :],
                                  xT_d[kc * 128:(kc + 1) * 128, tsl])
            ps_q = ps_pool.tile([DH, 512], FP32, name="ps_q", tag="ps")
            for kc in range(DCH):
                nc.tensor.matmul(ps_q[:], wq_sb[:, kc, :], xc[:, kc, :],
                                 start=(kc == 0), stop=(kc == DCH - 1))
            nc.scalar.copy(qT[:, tsl], ps_q[:])
            ps_k = ps_pool.tile([DH, 512], FP32, name="ps_k", tag="ps")
            for kc in range(DCH):
                nc.tensor.matmul(ps_k[:], wk_sb[:, kc, :], xc[:, kc, :],
                                 start=(kc == 0), stop=(kc == DCH - 1))
            nc.scalar.copy(kT[:, tsl], ps_k[:])
            # v token-major, 4 token-tiles packed per PSUM tile / copy
            ps_v = ps_pool.tile([128, 512], FP32, name="ps_v", tag="ps")
            for s in range(4):
                ssl = slice(s * 128, (s + 1) * 128)
                for kc in range(DCH):
                    nc.tensor.matmul(ps_v[:, ssl], xc[:, kc, ssl],
                                     wv_sb[:, kc, :],
                                     start=(kc == 0), stop=(kc == DCH - 1))
            nc.scalar.copy(
                vt[:, tcn * 4:(tcn + 1) * 4, :, 0:HD],
                ps_v[:].rearrange("p (t h d) -> p t h d", t=4, h=HPC))

    def emit_out(bb, pair):
        # two token-tiles per PSUM tile / copy (amortize overheads)
        it = 2 * pair
        ps_o = ps_pool.tile([128, 2, D], FP32, name="ps_o", tag="ps")
        for i in range(2):
            nc.tensor.matmul(
                ps_o[:, i, :],
                att_all[:, bb * V + (it + i) * 128:bb * V + (it + i + 1) * 128],
                woth[:], start=True, stop=True)
        o_sb = o_pool.tile([128, 2, D], BF16, name="o_sb", tag="o_sb")
        nc.vector.tensor_copy(o_sb[:], ps_o[:])
        for i in range(2):
            nc.sync.dma_start(out_d[bb, (it + i) * 128:(it + i + 1) * 128, :],
                              o_sb[:, i, :])

    emit_proj(0)
    for b in range(BPC):
        t0 = b * V
        if b + 1 < BPC:
            emit_proj(b + 1)
        ps_att = {}
        for h in range(HPC):
            ps_att[h] = ps_att_pool.tile([HD + 1, V], FP32,
                                         name=f"ps_att{h}", tag="ps_att")
        p2s = {}

        def emit_pv(jt, b=b, ps_att=ps_att, p2s=p2s):
            for h in range(HPC):
                p2 = p2s.pop((jt, h))
                for ic in range(2):
                    isl = slice(ic * 512, (ic + 1) * 512)
                    nc.tensor.matmul(ps_att[h][:, isl],
                                     vt[:, b * NJT + jt, h, :], p2[:, isl],
                                     start=(jt == 0), stop=(jt == NJT - 1))

        for jt in range(NJT):
            jsl = slice(t0 + jt * 128, t0 + (jt + 1) * 128)
            ps_sA = ps_pool.tile([128, V], FP32, name="ps_sA", tag="ps")
            ps_sB = ps_pool.tile([128, V], FP32, name="ps_sB", tag="ps")
            for sc in range(2):
                csl_ = slice(sc * 512, (sc + 1) * 512)
                ssl = slice(t0 + sc * 512, t0 + (sc + 1) * 512)
                nc.tensor.matmul(ps_sA[:, csl_], kT[0:HD, jsl], qT[0:HD, ssl],
                                 start=True, stop=True)
                nc.tensor.matmul(ps_sB[:, csl_], kT[HD:DH, jsl],
                                 qT[HD:DH, ssl], start=True, stop=True)
            for h, ps_s, w_t in ((0, ps_sA, w_A), (1, ps_sB, w_B)):
                p_sb = p_pool.tile([128, V], BF16, name="p_sb", tag="p")
                nc.scalar.activation(p_sb[:], ps_s[:], ACTF.Exp)
                p2 = p2_pool.tile([128, V], BF16, name="p2", tag="p2")
                nc.vector.tensor_tensor(p2[:], p_sb[:],
                                        w_t[:, jt * V:(jt + 1) * V],
                                        AOT.mult)
                p2s[(jt, h)] = p2
            # software-pipelined PV: consume the previous jt's P'
            if jt > 0:
                emit_pv(jt - 1)
            # out-proj of the previous batch rides the tail of this one
            if b > 0 and jt >= 4:
                emit_out(b - 1, jt - 4)
        emit_pv(NJT - 1)

        # ---- drain PSUM, then denominators + normalization ---------------
        for h in range(HPC):
            # single fast drain of the accumulated [att; den] tile to SBUF
            attr = attr_pool.tile([HD + 1, V], BF16, name="attr", tag="attr")
            nc.vector.tensor_copy(attr[:], ps_att[h][:])
            # den row -> DRAM -> [128, 8] -> reciprocal -> broadcast row
            dden = dram_rec.tile([1, V], BF16, name="dden", tag="dden")
            nc.sync.dma_start(dden[:], attr[HD:HD + 1, :])
            den128 = rec_pool.tile([128, 8], BF16, name="den128",
                                   tag="den128")
            nc.gpsimd.dma_start(
                den128[:], dden[:].rearrange("a (p c) -> (a p) c", p=128))
            den128f = rec_pool.tile([128, 8], FP32, name="den128f",
                                    tag="den128f")
            nc.vector.tensor_copy(den128f[:], den128[:])
            r128 = rec_pool.tile([128, 8], FP32, name="r128", tag="r128")
            nc.vector.reciprocal_approx_fast(r128[:], den128f[:])
            r128b = rec_pool.tile([128, 8], BF16, name="r128b", tag="r128b")
            nc.vector.tensor_copy(r128b[:], r128[:])
            rrec = dram_rec.tile([1, V], BF16, name="rrec", tag="rrec")
            nc.gpsimd.dma_start(
                rrec[:].rearrange("a (p c) -> (a p) c", p=128), r128b[:])
            rbc = rec_pool.tile([HD, V], BF16, name="rbc", tag="rbc")
            nc.gpsimd.dma_start(rbc[:], rrec[:].broadcast_to([HD, V]))
            nc.vector.tensor_tensor(
                att_all[h * HD:(h + 1) * HD, t0:t0 + V],
                attr[0:HD, :], rbc[:], AOT.mult)

    for pair in range(NJT // 2):
        emit_out(BPC - 1, pair)

    if dbg is not None:
        for nm, t in (("qT", qT), ("kT", kT), ("w_A", w_A), ("w_B", w_B),
                      ("att", att_all)):
            nslices = t.shape[1] // 4096 if t.shape[1] >= 4096 else 1
            for sidx in range(nslices):
                sl = slice(sidx * 4096, (sidx + 1) * 4096)
                nc.sync.dma_start(dbg[nm][:, sl], t[:, sl])

    ctx_att.close()
    ctx.close()


# --------------------------------------------------------------------------
# Host side
# --------------------------------------------------------------------------

def _bf16(a):
    import ml_dtypes
    return np.ascontiguousarray(a.astype(ml_dtypes.bfloat16))


def shard_inputs(x, Wq, Wk, Wv, Wo, bo, rpe, hop_matrix):
    x = np.asarray(x, np.float32)
    scale = 1.0 / np.float32(np.sqrt(HD))
    WqT = np.asarray(Wq, np.float32).T * scale
    WkT = np.asarray(Wk, np.float32).T
    WvT = np.asarray(Wv, np.float32).T
    WoT = np.asarray(Wo, np.float32).T
    hopT = np.asarray(hop_matrix).T          # [key, query] layout
    wtab = np.exp(np.asarray(rpe, np.float32))  # [H, NHOP]
    in_maps = []
    for c in range(N_CORES):
        hg, bg = c // BG, c % BG
        csl = slice(hg * DH, (hg + 1) * DH)
        xs = x[bg * BPC:(bg + 1) * BPC].reshape(T, D).T
        hA, hB = 2 * hg, 2 * hg + 1
        in_maps.append({
            "xT": _bf16(xs),
            "WqTc": _bf16(WqT[:, csl]),
            "WkTc": _bf16(WkT[:, csl]),
            "WvTc": _bf16(WvT[:, csl]),
            "WoTh": np.ascontiguousarray(WoT[csl, :]),
            "wA": _bf16(wtab[hA][hopT]),
            "wB": _bf16(wtab[hB][hopT]),
        })
    return in_maps


def unshard_output(results, bo):
    bo = np.asarray(bo, np.float32)
    outs = []
    for bg in range(BG):
        acc = np.zeros((BPC, V, D), np.float32)
        for hg in range(HG):
            acc += results[hg * BG + bg]["out"].astype(np.float32)
        outs.append(acc + bo)
    return np.concatenate(outs, axis=0)


_CACHE = {}


def _get_compiled():
    if "nc" in _CACHE:
        return _CACHE["nc"]
    nc = bacc.Bacc("TRN2", target_bir_lowering=False, debug=False,
                   num_devices=N_CORES)
    ins = {
        "xT": nc.dram_tensor("xT", [D, T], BF16, kind="ExternalInput").ap(),
        "WqTc": nc.dram_tensor("WqTc", [D, DH], BF16,
                               kind="ExternalInput").ap(),
        "WkTc": nc.dram_tensor("WkTc", [D, DH], BF16,
                               kind="ExternalInput").ap(),
        "WvTc": nc.dram_tensor("WvTc", [D, DH], BF16,
                               kind="ExternalInput").ap(),
        "WoTh": nc.dram_tensor("WoTh", [DH, D], FP32,
                               kind="ExternalInput").ap(),
        "wA": nc.dram_tensor("wA", [V, V], BF16, kind="ExternalInput").ap(),
        "wB": nc.dram_tensor("wB", [V, V], BF16, kind="ExternalInput").ap(),
    }
    out = nc.dram_tensor("out", [BPC, V, D], BF16,
                         kind="ExternalOutput").ap()
    import os
    dbg = None
    if os.environ.get("KBG_DEBUG"):
        dbg = {
            "qT": nc.dram_tensor("dbg_qT", [DH, T], BF16,
                                 kind="ExternalOutput").ap(),
            "kT": nc.dram_tensor("dbg_kT", [DH, T], BF16,
                                 kind="ExternalOutput").ap(),
            "w_A": nc.dram_tensor("dbg_w_A", [128, NJT * V], BF16,
                                  kind="ExternalOutput").ap(),
            "w_B": nc.dram_tensor("dbg_w_B", [128, NJT * V], BF16,
                                  kind="ExternalOutput").ap(),
            "att": nc.dram_tensor("dbg_att", [DH, T], BF16,
                                  kind="ExternalOutput").ap(),
        }
    with tile.TileContext(nc) as tc:
        build_graph(tc, out, ins, dbg)
    nc.compile()
    _CACHE["nc"] = nc
    return nc


def kernel(x, Wq, Wk, Wv, Wo, bo, rpe, hop_matrix):
    from concourse.bass_utils import run_bass_kernel_spmd

    nc = _get_compiled()
    in_maps = shard_inputs(x, Wq, Wk, Wv, Wo, bo, rpe, hop_matrix)
    res = run_bass_kernel_spmd(nc, in_maps, core_ids=list(range(N_CORES)))
    return unshard_output(res.results, bo)
